# revision 4
# baseline (speedup 1.0000x reference)
"""Dense vanilla attention (B=32, S=1024, D=512, H=64) on 8 NeuronCores.

Data-parallel over batch: each core gets 4 batches, full weights.
Per-core Bass/Tile kernel computes, per batch:
    mlpT   = relu(w1^T @ queryT + w1_b)            [H, S]
    qT     = q_w^T @ queryT + q_b                  [D, S]
    kT     = k_w^T @ keyT  + k_b                   [D, S]
    scores = qT^T @ kT + mlp_aug^T @ w2_aug        [S, S]  (per 128-row block)
    attn   = softmax(scores, axis=-1)  -> DRAM
    out    = (exp(scores-max) @ value) * 1/sum     -> DRAM
Matmuls run as float32r (~13-bit mantissa, 1 cyc/row on PE); activation
transposes via PE identity-matmul, 4 packed per PSUM bank.
"""

import numpy as np

B, S, D, H = 32, 1024, 512, 64
NCORES = 8
BPC = B // NCORES  # batches per core
KC = D // 128      # 4 contraction chunks of 128
NB = S // 128      # 8 row/col blocks of 128

_BUILD_CACHE = {}


def _build(nbatches=BPC):
    import concourse.mybir as mybir
    import concourse.tile as tile
    from concourse import bacc
    from bass_rust import add_dep_helper
    from contextlib import ExitStack

    dt = mybir.dt
    f32 = dt.float32
    f32r = dt.float32r
    AF = mybir.ActivationFunctionType
    AX = mybir.AxisListType

    nc = bacc.Bacc("TRN2", target_bir_lowering=False)

    query = nc.dram_tensor("query", [nbatches, S, D], f32, kind="ExternalInput")
    key = nc.dram_tensor("key", [nbatches, S, D], f32, kind="ExternalInput")
    value = nc.dram_tensor("value", [nbatches, S, D], f32, kind="ExternalInput")
    w1_w = nc.dram_tensor("w1_w", [D, H], f32, kind="ExternalInput")
    w1_b = nc.dram_tensor("w1_b", [H], f32, kind="ExternalInput")
    w2_w = nc.dram_tensor("w2_w", [H, S], f32, kind="ExternalInput")
    w2_b = nc.dram_tensor("w2_b", [S], f32, kind="ExternalInput")
    q_w = nc.dram_tensor("q_w", [D, D], f32, kind="ExternalInput")
    q_b = nc.dram_tensor("q_b", [D], f32, kind="ExternalInput")
    k_w = nc.dram_tensor("k_w", [D, D], f32, kind="ExternalInput")
    k_b = nc.dram_tensor("k_b", [D], f32, kind="ExternalInput")
    ident_in = nc.dram_tensor("ident", [128, 128], f32, kind="ExternalInput")
    ones_in = nc.dram_tensor("ones", [1, S], f32, kind="ExternalInput")
    out = nc.dram_tensor("out", [nbatches, S, D], f32, kind="ExternalOutput")
    attention = nc.dram_tensor("attention", [nbatches, S, S], f32, kind="ExternalOutput")

    def r(ap):
        return ap.bitcast(f32r)

    with tile.TileContext(nc) as tc, ExitStack() as ctx:
        consts = ctx.enter_context(tc.tile_pool(name="consts", bufs=1))
        io = ctx.enter_context(tc.tile_pool(name="io", bufs=3))
        work = ctx.enter_context(tc.tile_pool(name="work", bufs=1))
        ps = ctx.enter_context(tc.tile_pool(name="ps", bufs=8, space="PSUM"))

        ident = consts.tile([128, 128], f32r, name="ident")
        nc.sync.dma_start(out=ident, in_=r(ident_in[:, :]))

        qw_sb = consts.tile([128, KC, D], f32r, name="qw_sb")
        nc.sync.dma_start(out=qw_sb, in_=r(q_w.rearrange("(kc p) n -> p kc n", p=128)))
        kw_sb = consts.tile([128, KC, D], f32r, name="kw_sb")
        nc.sync.dma_start(out=kw_sb, in_=r(k_w.rearrange("(kc p) n -> p kc n", p=128)))
        w1_sb = consts.tile([128, KC, H], f32r, name="w1_sb")
        nc.sync.dma_start(out=w1_sb, in_=r(w1_w.rearrange("(kc p) h -> p kc h", p=128)))
        w2a_sb = consts.tile([H + 1, S], f32r, name="w2a_sb")
        nc.sync.dma_start(out=w2a_sb[:H, :], in_=r(w2_w[:, :]))
        nc.sync.dma_start(out=w2a_sb[H : H + 1, :], in_=r(w2_b[None, :]))
        qb_sb = consts.tile([128, KC], f32, name="qb_sb")
        nc.sync.dma_start(out=qb_sb, in_=q_b.rearrange("(kc p) -> p kc", p=128))
        kb_sb = consts.tile([128, KC], f32, name="kb_sb")
        nc.sync.dma_start(out=kb_sb, in_=k_b.rearrange("(kc p) -> p kc", p=128))
        w1b_sb = consts.tile([H, 1], f32, name="w1b_sb")
        nc.sync.dma_start(out=w1b_sb, in_=w1_b[:, None])

        def transpose_pack(dst, src_block):
            """PE-transpose 8 [128,128] sub-tiles of src_block into dst.

            src_block[p, n, c]: n indexes 8 row sub-blocks, c a 128-wide column
            chunk; dst [128, 1024] receives sub-block n at columns n*128.
            Packs 4 transposes per PSUM bank (disjoint columns of one
            accumulation group), then one copy per bank to SBUF.
            """
            for g in range(2):
                tp = ps.tile([128, 512], f32r, name="tp", tag="ps")
                prev = None
                for j in range(4):
                    n = g * 4 + j
                    mm = nc.tensor.matmul(
                        tp[:, j * 128 : (j + 1) * 128],
                        src_block[:, n, :],
                        ident,
                        is_transpose=True,
                        start=(j == 0),
                        stop=(j == 3),
                    )
                    if prev is not None:
                        add_dep_helper(mm.ins, prev.ins, sync=False,
                                       reason="psum zero-region write order")
                    prev = mm
                nc.any.tensor_copy(dst[:, g * 512 : (g + 1) * 512], tp)

        for b in range(nbatches):
            qn = io.tile([128, NB, D], f32r, name="qn", tag="qkv")
            nc.sync.dma_start(out=qn, in_=r(query[b].rearrange("(n p) d -> p n d", p=128)))
            kn = io.tile([128, NB, D], f32r, name="kn", tag="qkv")
            nc.sync.dma_start(out=kn, in_=r(key[b].rearrange("(n p) d -> p n d", p=128)))
            vn = io.tile([128, NB, D], f32r, name="vn", tag="qkv")
            nc.sync.dma_start(out=vn, in_=r(value[b].rearrange("(n p) d -> p n d", p=128)))

            # -------- transposed raw inputs: queryT/keyT [din, s] --------
            qTn = work.tile([128, KC, S], f32r, name="qTn", tag="tn", bufs=2)
            kTn = work.tile([128, KC, S], f32r, name="kTn", tag="tn", bufs=2)
            for kc in range(KC):
                transpose_pack(qTn[:, kc, :], qn[:, :, kc * 128 : (kc + 1) * 128])
            for kc in range(KC):
                transpose_pack(kTn[:, kc, :], kn[:, :, kc * 128 : (kc + 1) * 128])

            # -------- projections qT/kT [dout, s] (+bias) --------
            qT = work.tile([128, KC, S], f32r, name="qT", tag="proj", bufs=3)
            kT = work.tile([128, KC, S], f32r, name="kT", tag="proj", bufs=3)
            for dst, wsb, bsb, src in ((qT, qw_sb, qb_sb, qTn), (kT, kw_sb, kb_sb, kTn)):
                for mc in range(KC):
                    for g2 in range(2):
                        pps = ps.tile([128, 512], f32, name="pps", tag="ps")
                        for kc in range(KC):
                            nc.tensor.matmul(
                                pps,
                                wsb[:, kc, mc * 128 : (mc + 1) * 128],
                                src[:, kc, g2 * 512 : (g2 + 1) * 512],
                                start=(kc == 0),
                                stop=(kc == KC - 1),
                            )
                        nc.any.tensor_scalar_add(
                            dst[:, mc, g2 * 512 : (g2 + 1) * 512], pps,
                            bsb[:, mc : mc + 1],
                        )

            # -------- mlp_aug [H+1, S]: relu(w1^T @ queryT + w1_b); row H = 1 --------
            mlpa = work.tile([H + 1, S], f32r, name="mlpa", tag="mlp", bufs=2)
            nc.sync.dma_start(out=mlpa[H : H + 1, :], in_=r(ones_in[:, :]))
            for g2 in range(2):
                mps = ps.tile([H, 512], f32, name="mps", tag="ps")
                for kc in range(KC):
                    nc.tensor.matmul(
                        mps,
                        w1_sb[:, kc, :],
                        qTn[:, kc, g2 * 512 : (g2 + 1) * 512],
                        start=(kc == 0),
                        stop=(kc == KC - 1),
                    )
                nc.scalar.activation(
                    mlpa[:H, g2 * 512 : (g2 + 1) * 512], mps, AF.Relu,
                    bias=w1b_sb, scale=1.0,
                )

            # -------- per 128-row block: scores -> softmax -> AV --------
            for sb in range(NB):
                srow = slice(sb * 128, (sb + 1) * 128)
                sps = []
                for h in range(2):
                    sp = ps.tile([128, 512], f32, name="sp", tag="ps")
                    for kc in range(KC):
                        nc.tensor.matmul(
                            sp,
                            qT[:, kc, srow],
                            kT[:, kc, h * 512 : (h + 1) * 512],
                            start=(kc == 0),
                            stop=False,
                        )
                    nc.tensor.matmul(
                        sp,
                        mlpa[:, srow],
                        w2a_sb[:, h * 512 : (h + 1) * 512],
                        start=False,
                        stop=True,
                    )
                    sps.append(sp)

                st = work.tile([128, 8], f32, name="st", tag="st", bufs=6)
                nc.vector.reduce_max(st[:, 0:1], sps[0], axis=AX.X)
                nc.vector.reduce_max(st[:, 1:2], sps[1], axis=AX.X)
                nc.vector.tensor_max(st[:, 2:3], st[:, 0:1], st[:, 1:2])
                nc.vector.tensor_scalar_mul(st[:, 3:4], st[:, 2:3], -1.0)

                ea = work.tile([128, S], f32r, name="ea", tag="ea", bufs=2)
                for h in range(2):
                    nc.scalar.activation(
                        ea[:, h * 512 : (h + 1) * 512], sps[h], AF.Exp,
                        bias=st[:, 3:4], scale=1.0,
                        accum_out=st[:, 4 + h : 5 + h],
                    )
                nc.vector.tensor_add(st[:, 6:7], st[:, 4:5], st[:, 5:6])
                nc.vector.reciprocal(st[:, 7:8], st[:, 6:7])

                at = work.tile([128, S], f32, name="at", tag="at", bufs=2)
                nc.any.tensor_scalar_mul(at, ea, st[:, 7:8])
                nc.sync.dma_start(out=attention[b, srow, :], in_=at)

                # transpose unnormalized exp-scores for AV
                aT = []
                for g in range(2):
                    tps = ps.tile([128, 512], f32r, name="tps", tag="ps")
                    prev = None
                    for j in range(4):
                        t_i = g * 4 + j
                        mm = nc.tensor.matmul(
                            tps[:, j * 128 : (j + 1) * 128],
                            ea[:, t_i * 128 : (t_i + 1) * 128],
                            ident,
                            is_transpose=True,
                            start=(j == 0),
                            stop=(j == 3),
                        )
                        if prev is not None:
                            add_dep_helper(mm.ins, prev.ins, sync=False,
                                           reason="psum zero-region write order")
                        prev = mm
                    aTg = work.tile([128, 512], f32r, name="aTg", tag="aT", bufs=4)
                    nc.any.tensor_copy(aTg, tps)
                    aT.append(aTg)

                avp = ps.tile([128, 512], f32, name="avp", tag="ps")
                for t_i in range(NB):
                    g, j = divmod(t_i, 4)
                    nc.tensor.matmul(
                        avp,
                        aT[g][:, j * 128 : (j + 1) * 128],
                        vn[:, t_i, :],
                        start=(t_i == 0),
                        stop=(t_i == NB - 1),
                    )
                ob = work.tile([128, D], f32, name="ob", tag="ob", bufs=2)
                nc.scalar.activation(ob, avp, AF.Copy, bias=0.0, scale=st[:, 7:8])
                nc.sync.dma_start(out=out[b, srow, :], in_=ob)

    nc.compile()
    return nc


def _get_nc(nbatches=BPC):
    if nbatches not in _BUILD_CACHE:
        _BUILD_CACHE[nbatches] = _build(nbatches)
    return _BUILD_CACHE[nbatches]


def kernel(query, key, value, w1_w, w1_b, w2_w, w2_b, q_w, q_b, k_w, k_b,
           _trace=False):
    from concourse.bass_utils import run_bass_kernel_spmd

    nc = _get_nc(BPC)
    shared = {
        "w1_w": np.ascontiguousarray(np.asarray(w1_w, np.float32)),
        "w1_b": np.ascontiguousarray(np.asarray(w1_b, np.float32)),
        "w2_w": np.ascontiguousarray(np.asarray(w2_w, np.float32)),
        "w2_b": np.ascontiguousarray(np.asarray(w2_b, np.float32)),
        "q_w": np.ascontiguousarray(np.asarray(q_w, np.float32)),
        "q_b": np.ascontiguousarray(np.asarray(q_b, np.float32)),
        "k_w": np.ascontiguousarray(np.asarray(k_w, np.float32)),
        "k_b": np.ascontiguousarray(np.asarray(k_b, np.float32)),
        "ident": np.eye(128, dtype=np.float32),
        "ones": np.ones((1, S), dtype=np.float32),
    }
    query = np.ascontiguousarray(np.asarray(query, np.float32))
    key = np.ascontiguousarray(np.asarray(key, np.float32))
    value = np.ascontiguousarray(np.asarray(value, np.float32))
    in_maps = []
    for c in range(NCORES):
        sl = slice(c * BPC, (c + 1) * BPC)
        in_maps.append({
            "query": query[sl], "key": key[sl], "value": value[sl], **shared,
        })

    res = run_bass_kernel_spmd(nc, in_maps, list(range(NCORES)), trace=_trace)
    out = np.concatenate([r["out"] for r in res.results], axis=0)
    attn = np.concatenate([r["attention"] for r in res.results], axis=0)
    kernel.last_results = res
    return out, attn


# revision 5
# speedup vs baseline: 1.3563x; 1.3563x over previous
"""Dense vanilla attention (B=32, S=1024, D=512, H=64) on 8 NeuronCores.

Data-parallel over batch: each core gets 4 batches, full weights.
Per-core Bass/Tile kernel computes, per batch:
    mlpT   = relu(w1^T @ queryT + w1_b)            [H, S]
    qT     = q_w^T @ queryT + q_b                  [D, S]
    kT     = k_w^T @ keyT  + k_b                   [D, S]
    scores = qT^T @ kT + mlp_aug^T @ w2_aug        [S, S]  (per 128-row block)
    attn   = softmax(scores, axis=-1)  -> DRAM
    out    = (exp(scores-max) @ value) * 1/sum     -> DRAM
Matmuls run as float32r (~13-bit mantissa, 1 cyc/row on PE); activation
transposes via PE identity-matmul, 4 packed per PSUM bank.
"""

import numpy as np

B, S, D, H = 32, 1024, 512, 64
NCORES = 8
BPC = B // NCORES  # batches per core
KC = D // 128      # 4 contraction chunks of 128
NB = S // 128      # 8 row/col blocks of 128

_BUILD_CACHE = {}


def _build(nbatches=BPC):
    import concourse.mybir as mybir
    import concourse.tile as tile
    from concourse import bacc
    from bass_rust import add_dep_helper
    from contextlib import ExitStack

    dt = mybir.dt
    f32 = dt.float32
    f32r = dt.float32r
    AF = mybir.ActivationFunctionType
    AX = mybir.AxisListType

    nc = bacc.Bacc("TRN2", target_bir_lowering=False)

    query = nc.dram_tensor("query", [nbatches, S, D], f32, kind="ExternalInput")
    key = nc.dram_tensor("key", [nbatches, S, D], f32, kind="ExternalInput")
    value = nc.dram_tensor("value", [nbatches, S, D], f32, kind="ExternalInput")
    w1_w = nc.dram_tensor("w1_w", [D, H], f32, kind="ExternalInput")
    w1_b = nc.dram_tensor("w1_b", [H], f32, kind="ExternalInput")
    w2_w = nc.dram_tensor("w2_w", [H, S], f32, kind="ExternalInput")
    w2_b = nc.dram_tensor("w2_b", [S], f32, kind="ExternalInput")
    q_w = nc.dram_tensor("q_w", [D, D], f32, kind="ExternalInput")
    q_b = nc.dram_tensor("q_b", [D], f32, kind="ExternalInput")
    k_w = nc.dram_tensor("k_w", [D, D], f32, kind="ExternalInput")
    k_b = nc.dram_tensor("k_b", [D], f32, kind="ExternalInput")
    ident_in = nc.dram_tensor("ident", [128, 128], f32, kind="ExternalInput")
    ones_in = nc.dram_tensor("ones", [1, S], f32, kind="ExternalInput")
    out = nc.dram_tensor("out", [nbatches, S, D], f32, kind="ExternalOutput")
    attention = nc.dram_tensor("attention", [nbatches, S, S], f32, kind="ExternalOutput")

    def r(ap):
        return ap.bitcast(f32r)

    with tile.TileContext(nc) as tc, ExitStack() as ctx:
        consts = ctx.enter_context(tc.tile_pool(name="consts", bufs=1))
        io = ctx.enter_context(tc.tile_pool(name="io", bufs=3))
        work = ctx.enter_context(tc.tile_pool(name="work", bufs=1))
        ps = ctx.enter_context(tc.tile_pool(name="ps", bufs=8, space="PSUM"))

        ident = consts.tile([128, 128], f32r, name="ident")
        nc.sync.dma_start(out=ident, in_=r(ident_in[:, :]))

        qw_sb = consts.tile([128, KC, D], f32r, name="qw_sb")
        nc.sync.dma_start(out=qw_sb, in_=r(q_w.rearrange("(kc p) n -> p kc n", p=128)))
        kw_sb = consts.tile([128, KC, D], f32r, name="kw_sb")
        nc.sync.dma_start(out=kw_sb, in_=r(k_w.rearrange("(kc p) n -> p kc n", p=128)))
        w1_sb = consts.tile([128, KC, H], f32r, name="w1_sb")
        nc.sync.dma_start(out=w1_sb, in_=r(w1_w.rearrange("(kc p) h -> p kc h", p=128)))
        w2a_sb = consts.tile([H + 1, S], f32r, name="w2a_sb")
        nc.sync.dma_start(out=w2a_sb[:H, :], in_=r(w2_w[:, :]))
        nc.sync.dma_start(out=w2a_sb[H : H + 1, :], in_=r(w2_b[None, :]))
        qb_sb = consts.tile([128, KC], f32, name="qb_sb")
        nc.sync.dma_start(out=qb_sb, in_=q_b.rearrange("(kc p) -> p kc", p=128))
        kb_sb = consts.tile([128, KC], f32, name="kb_sb")
        nc.sync.dma_start(out=kb_sb, in_=k_b.rearrange("(kc p) -> p kc", p=128))
        w1b_sb = consts.tile([H, 1], f32, name="w1b_sb")
        nc.sync.dma_start(out=w1b_sb, in_=w1_b[:, None])

        def transpose_pack(dst, src_block):
            """PE-transpose 8 [128,128] sub-tiles of src_block into dst.

            src_block[p, n, c]: n indexes 8 row sub-blocks, c a 128-wide column
            chunk; dst [128, 1024] receives sub-block n at columns n*128.
            Packs 4 transposes per PSUM bank (disjoint columns of one
            accumulation group), then one copy per bank to SBUF.
            """
            for g in range(2):
                tp = ps.tile([128, 512], f32r, name="tp", tag="ps")
                prev = None
                for j in range(4):
                    n = g * 4 + j
                    mm = nc.tensor.matmul(
                        tp[:, j * 128 : (j + 1) * 128],
                        src_block[:, n, :],
                        ident,
                        is_transpose=True,
                        start=(j == 0),
                        stop=(j == 3),
                    )
                    if prev is not None:
                        add_dep_helper(mm.ins, prev.ins, sync=False,
                                       reason="psum zero-region write order")
                    prev = mm
                nc.any.tensor_copy(dst[:, g * 512 : (g + 1) * 512], tp)

        for b in range(nbatches):
            qn = io.tile([128, NB, D], f32r, name="qn", tag="qkv")
            nc.sync.dma_start(out=qn, in_=r(query[b].rearrange("(n p) d -> p n d", p=128)))
            kn = io.tile([128, NB, D], f32r, name="kn", tag="qkv")
            nc.sync.dma_start(out=kn, in_=r(key[b].rearrange("(n p) d -> p n d", p=128)))
            vn = io.tile([128, NB, D], f32r, name="vn", tag="qkv")
            nc.sync.dma_start(out=vn, in_=r(value[b].rearrange("(n p) d -> p n d", p=128)))

            # -------- transposed raw inputs: queryT/keyT [din, s] --------
            qTn = work.tile([128, KC, S], f32r, name="qTn", tag="tn", bufs=2)
            kTn = work.tile([128, KC, S], f32r, name="kTn", tag="tn", bufs=2)
            for kc in range(KC):
                transpose_pack(qTn[:, kc, :], qn[:, :, kc * 128 : (kc + 1) * 128])
            for kc in range(KC):
                transpose_pack(kTn[:, kc, :], kn[:, :, kc * 128 : (kc + 1) * 128])

            # -------- projections qT/kT [dout, s] (+bias) --------
            qT = work.tile([128, KC, S], f32r, name="qT", tag="proj", bufs=3)
            kT = work.tile([128, KC, S], f32r, name="kT", tag="proj", bufs=3)
            for dst, wsb, bsb, src in ((qT, qw_sb, qb_sb, qTn), (kT, kw_sb, kb_sb, kTn)):
                for mc in range(KC):
                    for g2 in range(2):
                        pps = ps.tile([128, 512], f32, name="pps", tag="ps")
                        for kc in range(KC):
                            nc.tensor.matmul(
                                pps,
                                wsb[:, kc, mc * 128 : (mc + 1) * 128],
                                src[:, kc, g2 * 512 : (g2 + 1) * 512],
                                start=(kc == 0),
                                stop=(kc == KC - 1),
                            )
                        nc.any.tensor_scalar_add(
                            dst[:, mc, g2 * 512 : (g2 + 1) * 512], pps,
                            bsb[:, mc : mc + 1],
                        )

            # -------- mlp_aug [H+1, S]: relu(w1^T @ queryT + w1_b); row H = 1 --------
            mlpa = work.tile([H + 1, S], f32r, name="mlpa", tag="mlp", bufs=2)
            nc.sync.dma_start(out=mlpa[H : H + 1, :], in_=r(ones_in[:, :]))
            for g2 in range(2):
                mps = ps.tile([H, 512], f32, name="mps", tag="ps")
                for kc in range(KC):
                    nc.tensor.matmul(
                        mps,
                        w1_sb[:, kc, :],
                        qTn[:, kc, g2 * 512 : (g2 + 1) * 512],
                        start=(kc == 0),
                        stop=(kc == KC - 1),
                    )
                nc.scalar.activation(
                    mlpa[:H, g2 * 512 : (g2 + 1) * 512], mps, AF.Relu,
                    bias=w1b_sb, scale=1.0,
                )

            # -------- per 128-row block: scores -> softmax -> AV --------
            # Software-pipelined: emit block sb's scores+softmax, then block
            # sb-1's transpose+AV consume stage, so the PE instruction stream
            # never waits on the softmax chain (keeps HAM warm).
            pending = None

            def produce(sb):
                srow = slice(sb * 128, (sb + 1) * 128)
                sps = []
                for h in range(2):
                    sp = ps.tile([128, 512], f32, name="sp", tag="ps")
                    for kc in range(KC):
                        nc.tensor.matmul(
                            sp,
                            qT[:, kc, srow],
                            kT[:, kc, h * 512 : (h + 1) * 512],
                            start=(kc == 0),
                            stop=False,
                        )
                    nc.tensor.matmul(
                        sp,
                        mlpa[:, srow],
                        w2a_sb[:, h * 512 : (h + 1) * 512],
                        start=False,
                        stop=True,
                    )
                    sps.append(sp)

                st = work.tile([128, 8], f32, name="st", tag="st", bufs=6)
                nc.vector.reduce_max(st[:, 0:1], sps[0], axis=AX.X)
                nc.vector.reduce_max(st[:, 1:2], sps[1], axis=AX.X)
                nc.vector.tensor_max(st[:, 2:3], st[:, 0:1], st[:, 1:2])
                nc.vector.tensor_scalar_mul(st[:, 3:4], st[:, 2:3], -1.0)

                ea = work.tile([128, S], f32r, name="ea", tag="ea", bufs=2)
                for h in range(2):
                    nc.scalar.activation(
                        ea[:, h * 512 : (h + 1) * 512], sps[h], AF.Exp,
                        bias=st[:, 3:4], scale=1.0,
                        accum_out=st[:, 4 + h : 5 + h],
                    )
                nc.vector.tensor_add(st[:, 6:7], st[:, 4:5], st[:, 5:6])
                nc.vector.reciprocal(st[:, 7:8], st[:, 6:7])

                at = work.tile([128, S], f32, name="at", tag="at", bufs=2)
                nc.any.tensor_scalar_mul(at, ea, st[:, 7:8])
                nc.sync.dma_start(out=attention[b, srow, :], in_=at)
                return (sb, ea, st)

            def consume(pend):
                sb, ea, st = pend
                srow = slice(sb * 128, (sb + 1) * 128)
                # transpose unnormalized exp-scores for AV
                aT = []
                for g in range(2):
                    tps = ps.tile([128, 512], f32r, name="tps", tag="ps")
                    prev = None
                    for j in range(4):
                        t_i = g * 4 + j
                        mm = nc.tensor.matmul(
                            tps[:, j * 128 : (j + 1) * 128],
                            ea[:, t_i * 128 : (t_i + 1) * 128],
                            ident,
                            is_transpose=True,
                            start=(j == 0),
                            stop=(j == 3),
                        )
                        if prev is not None:
                            add_dep_helper(mm.ins, prev.ins, sync=False,
                                           reason="psum zero-region write order")
                        prev = mm
                    aTg = work.tile([128, 512], f32r, name="aTg", tag="aT", bufs=4)
                    nc.any.tensor_copy(aTg, tps)
                    aT.append(aTg)

                avp = ps.tile([128, 512], f32, name="avp", tag="ps")
                for t_i in range(NB):
                    g, j = divmod(t_i, 4)
                    nc.tensor.matmul(
                        avp,
                        aT[g][:, j * 128 : (j + 1) * 128],
                        vn[:, t_i, :],
                        start=(t_i == 0),
                        stop=(t_i == NB - 1),
                    )
                ob = work.tile([128, D], f32, name="ob", tag="ob", bufs=2)
                nc.scalar.activation(ob, avp, AF.Copy, bias=0.0, scale=st[:, 7:8])
                nc.sync.dma_start(out=out[b, srow, :], in_=ob)

            for sb in range(NB):
                nxt = produce(sb)
                if pending is not None:
                    consume(pending)
                pending = nxt
            consume(pending)

    nc.compile()
    return nc


def _get_nc(nbatches=BPC):
    if nbatches not in _BUILD_CACHE:
        _BUILD_CACHE[nbatches] = _build(nbatches)
    return _BUILD_CACHE[nbatches]


def kernel(query, key, value, w1_w, w1_b, w2_w, w2_b, q_w, q_b, k_w, k_b,
           _trace=False):
    from concourse.bass_utils import run_bass_kernel_spmd

    nc = _get_nc(BPC)
    shared = {
        "w1_w": np.ascontiguousarray(np.asarray(w1_w, np.float32)),
        "w1_b": np.ascontiguousarray(np.asarray(w1_b, np.float32)),
        "w2_w": np.ascontiguousarray(np.asarray(w2_w, np.float32)),
        "w2_b": np.ascontiguousarray(np.asarray(w2_b, np.float32)),
        "q_w": np.ascontiguousarray(np.asarray(q_w, np.float32)),
        "q_b": np.ascontiguousarray(np.asarray(q_b, np.float32)),
        "k_w": np.ascontiguousarray(np.asarray(k_w, np.float32)),
        "k_b": np.ascontiguousarray(np.asarray(k_b, np.float32)),
        "ident": np.eye(128, dtype=np.float32),
        "ones": np.ones((1, S), dtype=np.float32),
    }
    query = np.ascontiguousarray(np.asarray(query, np.float32))
    key = np.ascontiguousarray(np.asarray(key, np.float32))
    value = np.ascontiguousarray(np.asarray(value, np.float32))
    in_maps = []
    for c in range(NCORES):
        sl = slice(c * BPC, (c + 1) * BPC)
        in_maps.append({
            "query": query[sl], "key": key[sl], "value": value[sl], **shared,
        })

    res = run_bass_kernel_spmd(nc, in_maps, list(range(NCORES)), trace=_trace)
    out = np.concatenate([r["out"] for r in res.results], axis=0)
    attn = np.concatenate([r["attention"] for r in res.results], axis=0)
    kernel.last_results = res
    return out, attn
